# revision 20
# baseline (speedup 1.0000x reference)
"""Batched SIR-ODE trajectory kernel for 8 Trainium2 NeuronCores.

Problem: params [65536, 4] = (beta, gamma, S0, I0) per sample ->
trajectories [65536, 200, 3] = (S, I, R) on the fixed 200-point grid,
matching the jax RK4(h) reference within 2e-2 relative.

Strategy (validated numerically to rel err ~2.1e-3 vs the reference):
  - integrate with RK4 at the DOUBLE step h' = 2h: 99 coarse steps cover
    the even grid points 0,2,...,198; one final fine h-step gives 199;
  - odd grid points come from cubic-Hermite interpolation using the
    states and stage-1 slopes at the coarse points:
      mid = 0.5*(y_n + y_{n+1}) - (h'/8)*(K_n - K_{n+1}),  K = -f.
    This halves the sequential RK4 chain and turns half of the output
    work into big batched off-critical-path ops.

Sharding: pure data parallel - core c integrates samples
[c*8192, (c+1)*8192). No cross-core communication.

Per-core engine plan:
  - DVE runs ONLY the serial chain: per coarse step 13 ops
    (4x [W-prod custom | W-diff | K=BG*W], 3x y-build STT,
     A=A123+K4, st' STT). A custom DVE op PROD_DIFF_SIR computes
    P = S*(T-S) in one instruction (state is packed (S, T=S+I)).
  - POOL (gpsimd) takes the off-critical work: A-chain partials
    (A12, A123), Hermite mid-point math, and the I = T-S output forms.
  - ACT (scalar) emits the S and R = 1-T output copies.
  - States/slopes land in SBUF ring buffers (st_ring, K_ring) written
    directly by the chain; interpolation reads lag the chain by one
    step; outputs stage into a double buffer DMA'd per 50-t chunk.

Build-level workarounds (kept from the previous iteration of this
kernel): single-sem-wait splitting onto NoOps, and stripping of Tile's
same-engine self-serialization semaphores (same-engine ordering is
already in-order; this is worth ~2x on the compute time).
"""
import bisect

import numpy as np

import concourse.bass as bass
import concourse.mybir as mybir
from concourse.tile import TileContext
from concourse.vector_clock import ScopedClock
import concourse.tile as tile_mod

F32 = mybir.dt.float32
ALU = mybir.AluOpType
ACTF = mybir.ActivationFunctionType

B = 65536
N_CORES = 8
N_PER_CORE = B // N_CORES  # 8192
N_T = 200
H = 100.0 / 199.0
H2 = 2.0 * H
NC_STEPS = 99          # coarse 2h steps: points 0..99 <-> t = 0,2,...,198
T_CHUNK = 20           # output time points per staged DMA chunk
M_CHUNK = 10           # coarse intervals per chunk (10 chunks)
N_CHUNKS = 10

# ---------------------------------------------------------------------------
# toolchain workarounds (unchanged from the previous kernel)
# ---------------------------------------------------------------------------


def _patched_drain_and_barrier(self, tick_clock, wait_clock):
    drain_inst = self.nc.sync.drain()
    wait_clock.add_sem_waits(
        drain_inst.ins, ScopedClock({None: tick_clock.global_clock})
    )
    si = drain_inst.ins.sync_info
    if si is not None and len(si.on_wait) > 1:
        waits = list(si.on_wait)
        upds = list(si.on_update)
        drain_inst.ins.sync_info = mybir.SyncInfo(on_wait=waits[:1], on_update=[])
        last = drain_inst
        for w in waits[1:]:
            last = self.nc.sync.drain()
            last.ins.sync_info = mybir.SyncInfo(on_wait=[w], on_update=[])
        if upds:
            cur = last.ins.sync_info
            last.ins.sync_info = mybir.SyncInfo(
                on_wait=list(cur.on_wait), on_update=upds
            )
    self.nc.all_engine_barrier()
    popped = self.nc._tile_sem_poison_stack.pop()
    assert popped is self._sem_poison
    self.nc.clear_and_free_semaphores(list(self.sems.allocated().values()))
    self.nc.all_engine_barrier()


tile_mod.TileContext._drain_and_barrier = _patched_drain_and_barrier

_split_cnt = [0]


def _split_multi_waits(nc):
    for fn in nc.m.functions:
        for bb in fn.blocks:
            insts = list(bb.instructions)
            out = []
            changed = False
            for inst in insts:
                si = getattr(inst, "sync_info", None)
                if si is not None and len(si.on_wait) > 1:
                    waits = list(si.on_wait)
                    for w in waits[:-1]:
                        _split_cnt[0] += 1
                        nop = mybir.InstNoOp(
                            name=f"wsplit-{_split_cnt[0]}", ins=[], outs=[]
                        )
                        nop.engine = inst.engine
                        nop.sync_info = mybir.SyncInfo(on_wait=[w], on_update=[])
                        out.append(nop)
                    inst.sync_info = mybir.SyncInfo(
                        on_wait=[waits[-1]], on_update=list(si.on_update)
                    )
                    changed = True
                out.append(inst)
            if changed:
                bb.instructions[:] = out


def _strip_self_sems(nc, engines=("DVE", "Pool", "Activation")):
    all_insts = []
    for fn in nc.m.functions:
        for bb in fn.blocks:
            for ins in bb.instructions:
                all_insts.append(ins)

    def ename(ins):
        return str(ins.engine).replace("EngineType.", "")

    inc_engines = {}
    wait_modes = {}
    for ins in all_insts:
        si = getattr(ins, "sync_info", None)
        if si is None:
            continue
        for u in si.on_update or []:
            if u.sync_type == "semaphore" and u.update_mode == "sem-inc":
                inc_engines.setdefault(u.id, set()).add(ename(ins))
            else:
                inc_engines.setdefault(u.id, set()).add("?" + str(u.update_mode))
        for w in si.on_wait or []:
            if w.sync_type == "semaphore":
                wait_modes.setdefault(w.id, set()).add(w.wait_mode)

    for eng in engines:
        sems = [
            sid
            for sid, engs in inc_engines.items()
            if engs == {eng}
            and all(m == "sem-ge-imm" for m in wait_modes.get(sid, set()))
        ]
        for sid in sems:
            waited = set()
            for ins in all_insts:
                si = getattr(ins, "sync_info", None)
                if si is None:
                    continue
                for w in si.on_wait or []:
                    if (
                        w.sync_type == "semaphore"
                        and w.id == sid
                        and ename(ins) != eng
                    ):
                        waited.add(w.wait_value)
            wl = sorted(waited)

            def nval(v):
                return bisect.bisect_right(wl, v)

            cum = 0
            for ins in all_insts:
                si = getattr(ins, "sync_info", None)
                if si is None:
                    continue
                ow = list(si.on_wait or [])
                ou = list(si.on_update or [])
                changed = False
                new_w = []
                for w in ow:
                    if w.sync_type == "semaphore" and w.id == sid:
                        changed = True
                        if ename(ins) == eng:
                            continue
                        new_w.append(
                            mybir.SyncWait(
                                ant_name=w.ant_name,
                                id=w.id,
                                sync_type=w.sync_type,
                                wait_mode=w.wait_mode,
                                wait_value=nval(w.wait_value),
                            )
                        )
                    else:
                        new_w.append(w)
                new_u = []
                for u in ou:
                    if (
                        u.sync_type == "semaphore"
                        and u.id == sid
                        and u.update_mode == "sem-inc"
                    ):
                        changed = True
                        lo = cum
                        cum += u.update_value
                        if any(lo < v <= cum for v in wl):
                            new_u.append(u)
                    else:
                        new_u.append(u)
                if changed:
                    ins.sync_info = mybir.SyncInfo(on_wait=new_w, on_update=new_u)


# ---------------------------------------------------------------------------
# custom DVE op: P = Src0 * (Src1 - Src0)  (= S*I when fed S and T = S+I)
# ---------------------------------------------------------------------------

_PD_OP = [None]


def _get_prod_diff_op():
    if _PD_OP[0] is not None:
        return _PD_OP[0]
    import concourse.dve_ops as dve_ops
    from concourse.dve_spec import Spec, Src0, Src1, lower
    from concourse.dve_uop import DveOpSpec

    name = "PROD_DIFF_SIR"
    if name in dve_ops._SUB_OPCODE_FOR_NAME:
        op = next(o for o in dve_ops.OPS if o.name == name)
        _PD_OP[0] = op
        return op
    spec = Spec(
        body=Src0 * (Src1 - Src0),
        reference=lambda in0, in1, s0, s1, imm2: (
            in0.astype(np.float32) * (in1.astype(np.float32) - in0.astype(np.float32))
        ),
    )
    row = dve_ops._CUSTOM_DVE_ROW_BASE + len(dve_ops.OPS)
    assert row < 0x20
    shas = {}
    for ver in ("v3", "v4"):
        tmp = DveOpSpec(
            name=name, opcode=row, uops=lower(spec, ver=ver), rd1_en=True
        )
        shas[ver] = tmp.sha(ver)
    op = dve_ops.DveOp(name, spec, subdim=False, uops_sha=shas)
    dve_ops.OPS.append(op)
    dve_ops._SUB_OPCODE_FOR_NAME[name] = row
    dve_ops.CUSTOM_DVE_SPECS[name] = spec
    _PD_OP[0] = op
    return op


# ---------------------------------------------------------------------------
# kernel build (per-core program; same NEFF runs SPMD on all 8 cores)
# ---------------------------------------------------------------------------


def _build():
    P = 128
    J = 64
    pd_op = _get_prod_diff_op()
    nc = bass.Bass(
        "TRN2", target_bir_lowering=False, debug=False, num_devices=N_CORES
    )
    params = nc.dram_tensor(
        "params", [N_PER_CORE, 4], F32, kind="ExternalInput"
    ).ap()
    out = nc.dram_tensor(
        "out", [N_PER_CORE, N_T, 3], F32, kind="ExternalOutput"
    ).ap()

    with TileContext(nc) as tc:
        with (
            tc.tile_pool(name="const", bufs=1) as cpool,
            tc.tile_pool(name="stage", bufs=2) as stpool,
        ):
            p4 = cpool.tile([P, J * 4], F32, tag="p4")
            nc.sync.dma_start(
                out=p4[:], in_=params.rearrange("(p j) q -> p (j q)", p=P)
            )
            p4r = p4.rearrange("p (j q) -> p j q", q=4)

            # Pre-scaled params:  bgt = (H2/4)*[beta|gamma], bgt2 = 2*bgt.
            # Slopes are stored as Kt = (H2/4)*K (stages 1/4) and
            # Kt' = (H2/2)*K (stages 2/3, pre-doubled), which makes
            #   - the A-chain  A = Kt1+Kt2'+Kt3'+Kt4  plain adds,
            #   - every y-build / final-update scalar a step-size-free
            #     constant (-2, -1, -2/3; tail: -1, -1/2, -1/3),
            #   - the Hermite mid  mid2 = u - (Kt_m - Kt_{m+1})  scale-free.
            bgt = cpool.tile([P, 2 * J], F32, tag="bgt")
            nc.vector.tensor_scalar_mul(bgt[:, 0:J], p4r[:, :, 0], H2 / 4)
            nc.vector.tensor_scalar_mul(bgt[:, J:], p4r[:, :, 1], H2 / 4)
            bgt2 = cpool.tile([P, 2 * J], F32, tag="bgt2")
            nc.vector.tensor_tensor(out=bgt2[:], in0=bgt[:], in1=bgt[:], op=ALU.add)

            # state / slope rings over the 100 coarse points
            st_ring = cpool.tile([P, 100 * 2 * J], F32, tag="st_ring")
            stv = st_ring.rearrange("p (n c) -> p n c", c=2 * J)
            k_ring = cpool.tile([P, 100 * 2 * J], F32, tag="k_ring")
            krv = k_ring.rearrange("p (n c) -> p n c", c=2 * J)

            # chain scratch (all DVE)
            wt = cpool.tile([P, 2 * J], F32, tag="wt")
            yt = cpool.tile([P, 2 * J], F32, tag="yt")
            k2 = cpool.tile([P, 2 * J], F32, tag="k2")
            k3 = cpool.tile([P, 2 * J], F32, tag="k3")
            k4 = cpool.tile([P, 2 * J], F32, tag="k4")
            aa1 = cpool.tile([P, 2 * J], F32, tag="aa1")
            aa2 = cpool.tile([P, 2 * J], F32, tag="aa2")
            aa = cpool.tile([P, 2 * J], F32, tag="aa")
            st199 = cpool.tile([P, 2 * J], F32, tag="st199")
            # batched interp scratch (POOL), one chunk of intervals at a time
            ut_b = cpool.tile([P, M_CHUNK * 2 * J], F32, tag="ut_b")
            vt_b = cpool.tile([P, M_CHUNK * 2 * J], F32, tag="vt_b")
            mt_b = cpool.tile([P, M_CHUNK * 2 * J], F32, tag="mt_b")
            i2_b = cpool.tile([P, M_CHUNK * J], F32, tag="i2_b")

            # initial state into ring slot 0: S, T = S0 + I0
            nc.vector.tensor_copy(out=stv[:, 0, 0:J], in_=p4r[:, :, 2])
            nc.vector.tensor_tensor(
                out=stv[:, 0, J:], in0=p4r[:, :, 2], in1=p4r[:, :, 3], op=ALU.add
            )

            def stage_w(y):
                """wt = [S*I | I] (I = T-S) from packed y, stock ops."""
                nc.vector.tensor_tensor(
                    out=wt[:, J:], in0=y[:, J:], in1=y[:, 0:J], op=ALU.subtract
                )
                nc.vector.tensor_tensor(
                    out=wt[:, 0:J], in0=y[:, 0:J], in1=wt[:, J:], op=ALU.mult
                )

            def stt(out_, in0, scalar, in1):
                nc.vector.scalar_tensor_tensor(
                    out=out_, in0=in0, scalar=scalar, in1=in1,
                    op0=ALU.mult, op1=ALU.add)

            def chain_step(st, k1_out, st_out, tail):
                """One RK4 step st -> st_out (coarse H2, or fine H if tail).

                Everything on DVE. Slopes pre-scaled via bgt/bgt2; the
                stage-1 Kt lands in k1_out (the K ring) for interpolation.
                """
                c_y2, c_y3, c_y4, c_f = (
                    (-1.0, -0.5, -1.0, -1.0 / 3.0) if tail else
                    (-2.0, -1.0, -2.0, -2.0 / 3.0)
                )
                stage_w(st)
                nc.vector.tensor_tensor(out=k1_out, in0=bgt[:], in1=wt[:], op=ALU.mult)
                if tail:
                    nc.vector.tensor_tensor(
                        out=yt[:], in0=st, in1=k1_out, op=ALU.subtract)
                else:
                    stt(yt[:], k1_out, c_y2, st)
                stage_w(yt)
                nc.vector.tensor_tensor(out=k2[:], in0=bgt2[:], in1=wt[:], op=ALU.mult)
                nc.vector.tensor_tensor(
                    out=aa1[:], in0=k1_out, in1=k2[:], op=ALU.add)
                if tail:
                    stt(yt[:], k2[:], c_y3, st)
                else:
                    nc.vector.tensor_tensor(
                        out=yt[:], in0=st, in1=k2[:], op=ALU.subtract)
                stage_w(yt)
                nc.vector.tensor_tensor(out=k3[:], in0=bgt2[:], in1=wt[:], op=ALU.mult)
                nc.vector.tensor_tensor(
                    out=aa2[:], in0=aa1[:], in1=k3[:], op=ALU.add)
                if tail:
                    nc.vector.tensor_tensor(
                        out=yt[:], in0=st, in1=k3[:], op=ALU.subtract)
                else:
                    stt(yt[:], k3[:], c_y4, st)
                stage_w(yt)
                nc.vector.tensor_tensor(out=k4[:], in0=bgt[:], in1=wt[:], op=ALU.mult)
                nc.vector.tensor_tensor(
                    out=aa[:], in0=aa2[:], in1=k4[:], op=ALU.add)
                stt(st_out, aa[:], c_f, st)

            # output staging double buffer: [j, t, q] per chunk of 50 t-points
            def new_stage(c):
                stg = stpool.tile([P, J * T_CHUNK * 3], F32, tag="stage",
                                  name=f"stg_{c}")
                return stg, stg.rearrange("p (j t q) -> p j t q", t=T_CHUNK, q=3)

            def emit_chunk(sgv, c):
                """All outputs for chunk c (t in [40c, 40c+40)), batched.

                Evens m in [20c, 20c+20) from the state ring; mids for
                intervals m in [20c, 20c+nm) via Hermite:
                mid2 = (y_m + y_{m+1}) - (Kt_m - Kt_{m+1}), outputs 0.5x.
                """
                m0 = c * M_CHUNK
                nm = M_CHUNK if c < N_CHUNKS - 1 else M_CHUNK - 1
                ne = M_CHUNK
                # ring views for this chunk, [p, n, half, j]
                sv = stv.rearrange("p n (h j) -> p n h j", h=2)
                m4 = mt_b.rearrange("p (n h j) -> p n h j", h=2, j=J)
                mv = mt_b.rearrange("p (n c) -> p n c", c=2 * J)
                uv = ut_b.rearrange("p (n c) -> p n c", c=2 * J)
                vv = vt_b.rearrange("p (n c) -> p n c", c=2 * J)
                iv = i2_b.rearrange("p (n j) -> p n j", j=J)
                # staging views iterated (n, j) to match the ring iteration
                s4 = sgv.rearrange("p j (tp two) q -> p tp two j q", two=2)
                s_even = s4[:, :, 0, :, :]
                s_odd = s4[:, :, 1, :, :]
                # --- evens (exact coarse states) ---
                nc.scalar.activation(
                    s_even[:, 0:ne, :, 0], sv[:, m0:m0 + ne, 0, :], ACTF.Copy,
                    bias=0.0, scale=1.0)
                nc.scalar.activation(
                    s_even[:, 0:ne, :, 2], sv[:, m0:m0 + ne, 1, :],
                    ACTF.Identity, bias=1.0, scale=-1.0)
                nc.gpsimd.tensor_tensor(
                    out=s_even[:, 0:ne, :, 1], in0=sv[:, m0:m0 + ne, 1, :],
                    in1=sv[:, m0:m0 + ne, 0, :], op=ALU.subtract)
                # --- mids ---
                nc.gpsimd.tensor_tensor(
                    out=uv[:, 0:nm, :], in0=stv[:, m0:m0 + nm, :],
                    in1=stv[:, m0 + 1:m0 + nm + 1, :], op=ALU.add)
                nc.gpsimd.tensor_tensor(
                    out=vv[:, 0:nm, :], in0=krv[:, m0:m0 + nm, :],
                    in1=krv[:, m0 + 1:m0 + nm + 1, :], op=ALU.subtract)
                nc.gpsimd.tensor_tensor(
                    out=mv[:, 0:nm, :], in0=uv[:, 0:nm, :],
                    in1=vv[:, 0:nm, :], op=ALU.subtract)  # mid2
                nc.scalar.activation(
                    s_odd[:, 0:nm, :, 0], m4[:, 0:nm, 0, :], ACTF.Copy,
                    bias=0.0, scale=0.5)
                nc.scalar.activation(
                    s_odd[:, 0:nm, :, 2], m4[:, 0:nm, 1, :], ACTF.Identity,
                    bias=1.0, scale=-0.5)
                nc.gpsimd.tensor_tensor(
                    out=iv[:, 0:nm, :], in0=m4[:, 0:nm, 1, :],
                    in1=m4[:, 0:nm, 0, :], op=ALU.subtract)
                nc.scalar.activation(
                    s_odd[:, 0:nm, :, 1], iv[:, 0:nm, :], ACTF.Copy,
                    bias=0.0, scale=0.5)

            def dma_chunk(sg, sgv, c):
                t0 = c * T_CHUNK
                nc.sync.dma_start(
                    out=out[:, t0:t0 + T_CHUNK, :].rearrange(
                        "(p j) t q -> p j (t q)", p=P),
                    in_=sgv.rearrange("p j t q -> p j (t q)"),
                )

            # chain, with chunk c's batched interp/output emitted as soon as
            # its last needed slope (Kt[10(c+1)]) exists
            stg, sgv = None, None
            for n in range(NC_STEPS):
                chain_step(stv[:, n, :], krv[:, n, :], stv[:, n + 1, :], False)
                # chunk c needs Kt[10c+10], written by step 10(c+1)'s stage 1
                if n > 0 and n % M_CHUNK == 0:
                    c = n // M_CHUNK - 1
                    stg, sgv = new_stage(c)
                    emit_chunk(sgv, c)
                    dma_chunk(stg, sgv, c)
            # tail: fine h-step from coarse point 99 -> t = 199; its stage-1
            # Kt is the slope at point 99 needed by interval 98's midpoint
            chain_step(stv[:, NC_STEPS, :], krv[:, NC_STEPS, :], st199[:], True)
            c = N_CHUNKS - 1
            stg, sgv = new_stage(c)
            emit_chunk(sgv, c)
            # t = 199 from st199 into the last t-slot
            nc.scalar.activation(
                sgv[:, :, T_CHUNK - 1, 0], st199[0:P, 0:J], ACTF.Copy,
                bias=0.0, scale=1.0)
            nc.scalar.activation(
                sgv[:, :, T_CHUNK - 1, 2], st199[0:P, J:], ACTF.Identity,
                bias=1.0, scale=-1.0)
            nc.gpsimd.tensor_tensor(
                out=sgv[:, :, T_CHUNK - 1, 1], in0=st199[0:P, J:],
                in1=st199[0:P, 0:J], op=ALU.subtract)
            dma_chunk(stg, sgv, c)

    _strip_self_sems(nc)
    _split_multi_waits(nc)
    # Encode .instr bytes for InstISA subclasses (the custom DVE op);
    # raw Bass skips this pass and the NEFF compiler then fails with
    # "ISA wrong length".
    mybir.codegen_inst_isa_subclasses(nc)
    return nc


# ---------------------------------------------------------------------------
# host entry: full inputs in, full output out, 8-core SPMD via PJRT
# ---------------------------------------------------------------------------

_CACHE = {}


def _get_runner():
    if "r" in _CACHE:
        return _CACHE["r"]
    import jax
    from jax.experimental.shard_map import shard_map
    from jax.sharding import Mesh, PartitionSpec

    from concourse.bass2jax import (
        _bass_exec_p,
        install_neuronx_cc_hook,
        partition_id_tensor,
    )

    install_neuronx_cc_hook()
    nc = _build()
    partition_name = nc.partition_id_tensor.name if nc.partition_id_tensor else None
    in_names, out_names, out_avals, zero_outs = [], [], [], []
    for alloc in nc.m.functions[0].allocations:
        if not isinstance(alloc, mybir.MemoryLocationSet):
            continue
        name = alloc.memorylocations[0].name
        if alloc.kind == "ExternalInput":
            if name != partition_name:
                in_names.append(name)
        elif alloc.kind == "ExternalOutput":
            shape = tuple(alloc.tensor_shape)
            dtype = mybir.dt.np(alloc.dtype)
            out_names.append(name)
            out_avals.append(jax.core.ShapedArray(shape, dtype))
            zero_outs.append(np.zeros(shape, dtype))

    def _body(*args):
        operands = list(args)
        if partition_name is not None:
            operands.append(partition_id_tensor())
        outs = _bass_exec_p.bind(
            *operands,
            out_avals=tuple(out_avals),
            in_names=tuple(
                in_names
                + out_names
                + ([partition_name] if partition_name else [])
            ),
            out_names=tuple(out_names),
            lowering_input_output_aliases=(),
            sim_require_finite=True,
            sim_require_nnan=True,
            nc=nc,
        )
        return tuple(outs)

    devices = jax.devices()[:N_CORES]
    mesh = Mesh(np.asarray(devices), ("core",))
    n_in = len(in_names)
    n_out = len(out_avals)
    fn = jax.jit(
        shard_map(
            _body,
            mesh=mesh,
            in_specs=(PartitionSpec("core"),) * (n_in + n_out),
            out_specs=(PartitionSpec("core"),) * n_out,
            check_rep=False,
        ),
        keep_unused=True,
    )
    _CACHE["r"] = (fn, in_names, out_names, out_avals, zero_outs, mesh)
    return _CACHE["r"]


def kernel(params: np.ndarray) -> np.ndarray:
    fn, in_names, out_names, out_avals, zero_outs, mesh = _get_runner()
    params = np.ascontiguousarray(np.asarray(params, dtype=np.float32))
    assert params.shape == (B, 4)
    ins = {"params": params}
    args = [ins[n] for n in in_names]
    args += [
        np.zeros((N_CORES * z.shape[0], *z.shape[1:]), z.dtype)
        for z in zero_outs
    ]
    outs = fn(*args)
    res = np.asarray(outs[out_names.index("out")])
    return res.reshape(B, N_T, 3)
